# revision 1
# baseline (speedup 1.0000x reference)
"""Trainium2 Bass kernel for nn_Memory_56513179681523 (product-key memory / retrieval KNN).

Computation (see problem reference):
  h  = elu(x @ W1.T + b1)                 # (B, 256)
  h2 = h @ W2.T + b2                      # (B, 1024)
  q  = batchnorm(h2) -> (B, 4 heads, 256) # training-mode batch stats
  per head: s1 = q1 @ K1.T, s2 = q2 @ K2.T   (512 sub-keys each, 128-dim halves)
  t1,i1 = top32(s1); t2,i2 = top32(s2)
  top32 of the 32x32 cartesian sums -> softmax weights w, flat idx = i1*512+i2
  out = sum_k w_k * values[idx_k]         # weighted EmbeddingBag over 512MB table

Strategy: data-parallel over batch across 8 cores, values table + MLP weights
replicated.  Every core computes the full-batch MLP (needed for exact BN batch
stats, avoids collectives) but keeps/normalizes/scores only its own 512-sample
slice.  Top-k via DVE max8/max_index/match_replace rounds; positional one-hot
gather recovers sub-key indices; weighted bag via per-sample PE matmuls over
indirect-DMA-gathered table rows.  Output written transposed, host re-transposes.
"""
import sys

try:
    import concourse.bass as bass  # noqa: F401
except Exception:  # pragma: no cover
    for p in ("/opt/trn_rl_repo", "/root/.axon_site/_ro/trn_rl_repo"):
        if p not in sys.path:
            sys.path.append(p)

import numpy as np
import concourse.bass as bass
import concourse.bacc as bacc
import concourse.mybir as mybir
import concourse.tile as tile
from concourse import bass_utils

F32 = mybir.dt.float32
U16 = mybir.dt.uint16
I32 = mybir.dt.int32
AX = mybir.AxisListType
OP = mybir.AluOpType
AF = mybir.ActivationFunctionType

P = 128            # partitions
HID = 512          # model dim / value row dim
F1 = 256           # first MLP layer out
F2 = 1024          # second MLP layer out
HEADS = 4
HALF = 128         # half key dim
SUB = 512          # sub-keys per half
KNN = 32
MEM = SUB * SUB    # value table rows
BN_EPS = 1e-5
NCORES = 8
NEG = -1e30


def build_program(b_tot: int, b_own: int, cc_cores: int = NCORES, debug_taps: bool = False):
    """Build the SPMD Bass program. b_tot = full batch, b_own = per-core slice."""
    assert b_tot % 512 == 0 and b_own % P == 0
    n_chunk = b_tot // 512          # MLP batch chunks (512 cols each)
    n_sc = b_own // P               # sample chunks of 128 for scoring/topk
    assert b_own <= 512

    nc = bass.Bass()

    xT = nc.dram_tensor("xT", [HID, b_own], F32, kind="ExternalInput")
    st_in = nc.dram_tensor("st_in", [P, 2 * F2 // P], F32)
    st_out = nc.dram_tensor("st_out", [P, 2 * F2 // P], F32)
    w1t = nc.dram_tensor("w1t", [HID, F1], F32, kind="ExternalInput")
    w2t = nc.dram_tensor("w2t", [F1, F2], F32, kind="ExternalInput")
    b1c = nc.dram_tensor("b1c", [P, F1 // P], F32, kind="ExternalInput")
    gamc = nc.dram_tensor("gamc", [P, F2 // P], F32, kind="ExternalInput")
    betc = nc.dram_tensor("betc", [P, F2 // P], F32, kind="ExternalInput")
    keysT = nc.dram_tensor("keysT", [2 * HEADS, HALF, SUB], F32, kind="ExternalInput")
    vals = nc.dram_tensor("vals", [MEM, HID], F32, kind="ExternalInput")
    iota32 = nc.dram_tensor("iota32", [P, KNN], U16, kind="ExternalInput")
    ident = nc.dram_tensor("ident", [P, P], F32, kind="ExternalInput")
    outT = nc.dram_tensor("outT", [HID, b_own], F32, kind="ExternalOutput")
    dbg = {}
    if debug_taps:
        dbg["sums"] = nc.dram_tensor("d_sums", [P, 16], F32, kind="ExternalOutput")
        dbg["gsums"] = nc.dram_tensor("d_gsums", [P, 16], F32, kind="ExternalOutput")
        dbg["qt0"] = nc.dram_tensor("d_qt0", [P, b_own], F32, kind="ExternalOutput")
        dbg["v0"] = nc.dram_tensor("d_v0", [P, KNN], F32, kind="ExternalOutput")
        dbg["pu0"] = nc.dram_tensor("d_pu0", [P, KNN], U16, kind="ExternalOutput")
        dbg["vS"] = nc.dram_tensor("d_vS", [P, KNN], F32, kind="ExternalOutput")
        dbg["pS"] = nc.dram_tensor("d_pS", [P, KNN], U16, kind="ExternalOutput")
        dbg["idxt"] = nc.dram_tensor("d_idxt", [P, P], I32, kind="ExternalOutput")
        dbg["wt"] = nc.dram_tensor("d_wt", [P, P], F32, kind="ExternalOutput")
        dbg["gb"] = nc.dram_tensor("d_gb", [P, 8 * HID], F32, kind="ExternalOutput")

    M1 = F1 // P   # 2
    M2 = F2 // P   # 8
    K1 = HID // P  # 4

    with tile.TileContext(nc) as tc:
        with (
            tc.tile_pool(name="const", bufs=1) as cpool,
            tc.tile_pool(name="persist", bufs=1) as ppool,
            tc.tile_pool(name="work", bufs=2) as wpool,
            tc.tile_pool(name="small", bufs=3) as spool,
        ):
            # ---- constants ----
            w1t_t = []
            for k in range(K1):
                t = cpool.tile([P, F1], F32, tag=f"w1t{k}")
                nc.sync.dma_start(t[:], w1t[k * P:(k + 1) * P, :])
                w1t_t.append(t)
            w2t_t = []
            for k in range(M1):
                t = cpool.tile([P, F2], F32, tag=f"w2t{k}")
                nc.sync.dma_start(t[:], w2t[k * P:(k + 1) * P, :])
                w2t_t.append(t)
            b1_t = cpool.tile([P, M1], F32, tag="b1")
            nc.sync.dma_start(b1_t[:], b1c[:])
            gam_t = cpool.tile([P, M2], F32, tag="gam")
            nc.sync.dma_start(gam_t[:], gamc[:])
            bet_t = cpool.tile([P, M2], F32, tag="bet")
            nc.sync.dma_start(bet_t[:], betc[:])
            iota_t = cpool.tile([P, KNN], U16, tag="iota")
            nc.sync.dma_start(iota_t[:], iota32[:])
            id_t = cpool.tile([P, P], F32, tag="ident")
            nc.sync.dma_start(id_t[:], ident[:])
            keys_t = []
            for j in range(2 * HEADS):
                t = cpool.tile([P, SUB], F32, tag=f"keys{j}")
                nc.sync.dma_start(t[:], keysT[j])
                keys_t.append(t)

            # ---- persistent state ----
            h2own = [ppool.tile([P, b_own], F32, name=f"h2own{m}", tag=f"h2own{m}") for m in range(M2)]
            qT = [ppool.tile([P, b_own], F32, name=f"qT{m}", tag=f"qT{m}") for m in range(M2)]

            # ================= Phase A: own-slice MLP + cross-core BN stats =================
            sums = ppool.tile([P, 2 * M2], F32, name="sums", tag="sums")
            with tc.tile_pool(name="mlp_ps", bufs=2, space="PSUM") as mlp_ps:
                xt_t = []
                for k in range(K1):
                    t = wpool.tile([P, b_own], F32, tag=f"xt{k}")
                    nc.sync.dma_start(t[:], xT[k * P:(k + 1) * P, :])
                    xt_t.append(t)
                h1_t = []
                for m in range(M1):
                    ps1 = mlp_ps.tile([P, b_own], F32, tag="ps1")
                    for k in range(K1):
                        nc.tensor.matmul(
                            ps1[:], lhsT=w1t_t[k][:, m * P:(m + 1) * P],
                            rhs=xt_t[k][:], start=(k == 0), stop=(k == K1 - 1))
                    e_t = wpool.tile([P, b_own], F32, tag="elu_e")
                    r_t = wpool.tile([P, b_own], F32, tag="elu_r")
                    nc.scalar.activation(out=e_t[:], in_=ps1[:], func=AF.Exp,
                                         bias=b1_t[:, m:m + 1], scale=1.0)
                    nc.scalar.activation(out=r_t[:], in_=ps1[:], func=AF.Relu,
                                         bias=b1_t[:, m:m + 1], scale=1.0)
                    h1 = wpool.tile([P, b_own], F32, tag=f"h1_{m}")
                    # elu(x) = min(exp(x) - 1, relu(x))
                    nc.vector.scalar_tensor_tensor(
                        out=h1[:], in0=e_t[:], scalar=1.0, in1=r_t[:],
                        op0=OP.subtract, op1=OP.min)
                    h1_t.append(h1)
                for m in range(M2):
                    ps2 = mlp_ps.tile([P, b_own], F32, tag="ps2")
                    for k in range(M1):
                        nc.tensor.matmul(
                            ps2[:], lhsT=w2t_t[k][:, m * P:(m + 1) * P],
                            rhs=h1_t[k][:], start=(k == 0), stop=(k == M1 - 1))
                    nc.scalar.copy(out=h2own[m][:], in_=ps2[:])
                    # partial sums for BN batch stats: sum(x), sum(x^2)
                    nc.vector.tensor_reduce(out=sums[:, 2 * m:2 * m + 1],
                                            in_=h2own[m][:], axis=AX.X, op=OP.add)
                    sq_scr = wpool.tile([P, b_own], F32, tag="sq_scr")
                    nc.scalar.activation(out=sq_scr[:], in_=h2own[m][:],
                                         func=AF.Square,
                                         accum_out=sums[:, 2 * m + 1:2 * m + 2])
            # 8-core all-reduce of the 16KB stats vector
            nc.sync.dma_start(st_in[:], sums[:])
            nc.gpsimd.collective_compute(
                "AllReduce", OP.add, replica_groups=[list(range(cc_cores))],
                ins=[st_in[:]], outs=[st_out[:]])
            gsums = ppool.tile([P, 2 * M2], F32, name="gsums", tag="gsums")
            nc.sync.dma_start(gsums[:], st_out[:])
            if debug_taps:
                nc.sync.dma_start(dbg["sums"][:], sums[:])
                nc.sync.dma_start(dbg["gsums"][:], gsums[:])

            # ---- BN finalize: mu = S1/N, var = S2/N - mu^2 ----
            eps_t = cpool.tile([P, 1], F32, tag="eps")
            nc.vector.memset(eps_t[:], float(BN_EPS))
            rn = 1.0 / float(b_tot)
            for m in range(M2):
                mu = spool.tile([P, 1], F32, name=f"mu{m}", tag="mu")
                nc.vector.tensor_scalar_mul(mu[:], gsums[:, 2 * m:2 * m + 1], rn)
                ex2 = spool.tile([P, 1], F32, name=f"ex2{m}", tag="ex2")
                nc.vector.tensor_scalar_mul(ex2[:], gsums[:, 2 * m + 1:2 * m + 2], rn)
                mu2 = spool.tile([P, 1], F32, name=f"mu2{m}", tag="mu2")
                nc.vector.tensor_tensor(out=mu2[:], in0=mu[:], in1=mu[:], op=OP.mult)
                var = spool.tile([P, 1], F32, name=f"var{m}", tag="var")
                nc.vector.tensor_tensor(out=var[:], in0=ex2[:], in1=mu2[:],
                                        op=OP.subtract)
                sq = spool.tile([P, 1], F32, name=f"sq{m}", tag="sq")
                nc.scalar.activation(out=sq[:], in_=var[:], func=AF.Sqrt,
                                     bias=eps_t[:], scale=1.0)
                inv = spool.tile([P, 1], F32, name=f"inv{m}", tag="inv")
                nc.vector.reciprocal(out=inv[:], in_=sq[:])
                scl = spool.tile([P, 1], F32, name=f"scl{m}", tag="scl")
                nc.vector.tensor_tensor(out=scl[:], in0=inv[:],
                                        in1=gam_t[:, m:m + 1], op=OP.mult)
                tmp = spool.tile([P, 1], F32, name=f"tmp{m}", tag="tmp")
                nc.vector.tensor_tensor(out=tmp[:], in0=mu[:], in1=scl[:],
                                        op=OP.mult)
                sh = spool.tile([P, 1], F32, name=f"sh{m}", tag="sh")
                nc.vector.tensor_tensor(out=sh[:], in0=bet_t[:, m:m + 1],
                                        in1=tmp[:], op=OP.subtract)
                nc.vector.tensor_scalar(out=qT[m][:, :b_own], in0=h2own[m][:, :b_own],
                                        scalar1=scl[:], scalar2=sh[:],
                                        op0=OP.mult, op1=OP.add)

            # ================= Phase B: score / topk / gather / bag =================
            with (
                tc.tile_pool(name="sc_ps", bufs=1, space="PSUM") as sc_ps,
                tc.tile_pool(name="tp_ps", bufs=2, space="PSUM") as tp_ps,
                tc.tile_pool(name="bag_ps", bufs=1, space="PSUM") as bag_ps,
                tc.tile_pool(name="stage", bufs=3) as stg,
                tc.tile_pool(name="gath", bufs=3) as gpool,
            ):
                tiles = [(sc, h) for sc in range(n_sc) for h in range(HEADS)]
                state = {}
                sc_state = {}

                def stage_a(t):
                    """Scores + stage-1/stage-2 topk rounds (DVE-chain heavy)."""
                    sc, h = tiles[t]
                    if h == 0:
                        sc_state[sc] = (
                            stg.tile([P, HEADS * KNN], F32,
                                     name=f"idx_all{sc}", tag="idx_all"),
                            stg.tile([P, HEADS * KNN], F32,
                                     name=f"w_all{sc}", tag="w_all"))
                    st = {}
                    for j in range(2):
                        ps = sc_ps.tile([P, SUB], F32, name=f"ps{j}_{t}",
                                        tag=f"score{j}")
                        nc.tensor.matmul(
                            ps[:],
                            lhsT=qT[2 * h + j][:, sc * P:(sc + 1) * P],
                            rhs=keys_t[2 * h + j][:], start=True, stop=True)
                        s_t = stg.tile([P, SUB], F32, name=f"s{j}_{t}", tag=f"s{j}")
                        nc.scalar.copy(out=s_t[:], in_=ps[:])
                        v_t = stg.tile([P, KNN], F32, name=f"v{j}_{t}", tag=f"v{j}")
                        pu_t = stg.tile([P, KNN], U16, name=f"pu{j}_{t}", tag=f"pu{j}")
                        for r in range(KNN // 8):
                            sl = slice(r * 8, (r + 1) * 8)
                            nc.vector.max(out=v_t[:, sl], in_=s_t[:])
                            nc.vector.max_index(out=pu_t[:, sl], in_max=v_t[:, sl],
                                                in_values=s_t[:])
                            nc.vector.match_replace(out=s_t[:], in_to_replace=v_t[:, sl],
                                                    in_values=s_t[:], imm_value=NEG)
                        st[f"v{j}"] = v_t
                        st[f"pu{j}"] = pu_t

                    # all_s[s, a*32+b] = t1[s,a] + t2[s,b]  (on DVE: keeps the
                    # topk dependency chain on one engine, no mid-chain stall)
                    as_t = stg.tile([P, KNN * KNN], F32, name=f"as_{t}", tag="all_s")
                    v1, v2 = st["v0"], st["v1"]
                    in0 = bass.AP(tensor=v1.tensor, offset=v1[:].offset,
                                  ap=[v1[:].ap[0], [1, KNN], [0, KNN]])
                    in1 = bass.AP(tensor=v2.tensor, offset=v2[:].offset,
                                  ap=[v2[:].ap[0], [0, KNN], [1, KNN]])
                    nc.vector.tensor_tensor(
                        out=as_t[:].rearrange("p (a b) -> p a b", a=KNN),
                        in0=in0, in1=in1, op=OP.add)

                    vS = stg.tile([P, KNN], F32, name=f"vS_{t}", tag="vS")
                    pS = stg.tile([P, KNN], U16, name=f"pS_{t}", tag="pS")
                    for r in range(KNN // 8):
                        sl = slice(r * 8, (r + 1) * 8)
                        nc.vector.max(out=vS[:, sl], in_=as_t[:])
                        nc.vector.max_index(out=pS[:, sl], in_max=vS[:, sl],
                                            in_values=as_t[:])
                        nc.vector.match_replace(out=as_t[:], in_to_replace=vS[:, sl],
                                                in_values=as_t[:], imm_value=NEG)
                    st["vS"] = vS
                    st["pS"] = pS
                    state[t] = st
                    if debug_taps and t == 0:
                        nc.sync.dma_start(dbg["qt0"][:], qT[0][:, :b_own])
                        nc.sync.dma_start(dbg["v0"][:], st["v0"][:])
                        nc.sync.dma_start(dbg["pu0"][:], st["pu0"][:])
                        nc.sync.dma_start(dbg["vS"][:], vS[:])
                        nc.sync.dma_start(dbg["pS"][:], pS[:])

                def stage_b(t):
                    """Softmax weights + index extraction (Pool/ACT heavy)."""
                    sc, h = tiles[t]
                    idx_all, w_all = sc_state[sc]
                    st = state.pop(t)
                    vS, pS = st["vS"], st["pS"]
                    mneg = spool.tile([P, 1], F32, name=f"mneg{t}", tag="mneg")
                    nc.vector.tensor_scalar_mul(mneg[:], vS[:, 0:1], -1.0)
                    ex = stg.tile([P, KNN], F32, name=f"ex{t}", tag="ex")
                    nc.scalar.activation(out=ex[:], in_=vS[:], func=AF.Exp,
                                         bias=mneg[:], scale=1.0)
                    zz = spool.tile([P, 1], F32, name=f"zz{t}", tag="zz")
                    nc.vector.tensor_reduce(out=zz[:], in_=ex[:], axis=AX.X,
                                            op=OP.add)
                    rz = spool.tile([P, 1], F32, name=f"rz{t}", tag="rz")
                    nc.vector.reciprocal(out=rz[:], in_=zz[:])
                    nc.vector.tensor_scalar(
                        out=w_all[:, h * KNN:(h + 1) * KNN], in0=ex[:],
                        scalar1=rz[:], scalar2=None, op0=OP.mult)

                    au = spool.tile([P, KNN], U16, name=f"au{t}", tag="au")
                    nc.vector.tensor_scalar(au[:], pS[:], 5, scalar2=None,
                                            op0=OP.logical_shift_right)
                    bu = spool.tile([P, KNN], U16, name=f"bu{t}", tag="bu")
                    nc.vector.tensor_scalar(bu[:], pS[:], 31, scalar2=None,
                                            op0=OP.bitwise_and)
                    isel = []
                    for j, abu in ((0, au), (1, bu)):
                        pf = spool.tile([P, KNN], F32, name=f"pf{j}_{t}", tag=f"pf{j}")
                        nc.vector.tensor_copy(out=pf[:], in_=st[f"pu{j}"][:])
                        eq = stg.tile([P, KNN * KNN], F32, name=f"eq{j}_{t}", tag="eq")
                        eq3 = eq[:].rearrange("p (j a) -> p j a", j=KNN)
                        in0e = bass.AP(tensor=abu.tensor, offset=abu[:].offset,
                                       ap=[abu[:].ap[0], [1, KNN], [0, KNN]])
                        in1e = bass.AP(tensor=iota_t.tensor, offset=iota_t[:].offset,
                                       ap=[iota_t[:].ap[0], [0, KNN], [1, KNN]])
                        nc.vector.tensor_tensor(out=eq3, in0=in0e, in1=in1e,
                                                op=OP.is_equal)
                        in1p = bass.AP(tensor=pf.tensor, offset=pf[:].offset,
                                       ap=[pf[:].ap[0], [0, KNN], [1, KNN]])
                        nc.vector.tensor_tensor(out=eq3, in0=eq3, in1=in1p,
                                                op=OP.mult)
                        sl_t = spool.tile([P, KNN], F32, name=f"isel{j}_{t}",
                                          tag=f"isel{j}")
                        nc.vector.tensor_reduce(out=sl_t[:], in_=eq3, axis=AX.X,
                                                op=OP.add)
                        isel.append(sl_t)
                    nc.vector.scalar_tensor_tensor(
                        out=idx_all[:, h * KNN:(h + 1) * KNN],
                        in0=isel[0][:], scalar=float(SUB), in1=isel[1][:],
                        op0=OP.mult, op1=OP.add)

                def bag(sc):
                    """Transpose to slot-major, gather table rows, weighted sum."""
                    idx_all, w_all = sc_state.pop(sc)
                    tp1 = tp_ps.tile([P, P], F32, name=f"tp1_{sc}", tag="tp")
                    nc.tensor.transpose(out=tp1[:], in_=idx_all[:], identity=id_t[:])
                    idxT = stg.tile([P, P], I32, name=f"idxT{sc}", tag="idxT")
                    nc.vector.tensor_copy(out=idxT[:], in_=tp1[:])
                    tp2 = tp_ps.tile([P, P], F32, name=f"tp2_{sc}", tag="tp")
                    nc.tensor.transpose(out=tp2[:], in_=w_all[:], identity=id_t[:])
                    wT = stg.tile([P, P], F32, name=f"wT{sc}", tag="wT")
                    nc.vector.tensor_copy(out=wT[:], in_=tp2[:])
                    if debug_taps and sc == 0:
                        nc.sync.dma_start(dbg["idxt"][:], idxT[:])
                        nc.sync.dma_start(dbg["wt"][:], wT[:])

                    # HW indirect DMA honors one offset per partition per
                    # instruction: gather each sample's 128 rows separately.
                    obag = [bag_ps.tile([P, P], F32, name=f"obag{c}_{sc}",
                                        tag=f"obag{c}") for c in range(4)]
                    for smp in range(P):
                        gb = gpool.tile([P, HID], F32, name=f"gb{smp}_{sc}",
                                        tag="gbuf", bufs=6)
                        nc.gpsimd.indirect_dma_start(
                            out=gb[:], out_offset=None, in_=vals[:],
                            in_offset=bass.IndirectOffsetOnAxis(
                                ap=idxT[:, smp:smp + 1], axis=0))
                        if debug_taps and sc == 0 and smp == 0:
                            nc.sync.dma_start(dbg["gb"][:, :HID], gb[:])
                        for c in range(4):
                            nc.tensor.matmul(
                                obag[c][:, smp:smp + 1],
                                lhsT=gb[:, c * P:(c + 1) * P],
                                rhs=wT[:, smp:smp + 1], start=True, stop=True)
                    for c in range(4):
                        osb = stg.tile([P, P], F32, name=f"osb{c}_{sc}", tag="osb")
                        nc.vector.tensor_copy(out=osb[:], in_=obag[c][:])
                        nc.sync.dma_start(
                            out=outT[c * P:(c + 1) * P, sc * P:(sc + 1) * P],
                            in_=osb[:])

                nt = len(tiles)
                for t in range(nt):
                    stage_a(t)
                    if t >= 1:
                        stage_b(t - 1)
                        if tiles[t - 1][1] == HEADS - 1:
                            bag(tiles[t - 1][0])
                stage_b(nt - 1)
                bag(n_sc - 1)
    return nc


def _split_matmul_waits(nc):
    """TRN2 instructions carry at most one sync wait; split excess waits onto
    InstEventSemaphore carriers (the same legalization Bacc.compile runs)."""
    import bass_rust as _br
    _br.generate_event_semaphores(nc)


def stage_inputs(inputs, b_tot: int, b_own: int, n_cores: int):
    """Host-side staging: transposes, constants, per-core batch rotation."""
    x = np.asarray(inputs["x"], np.float32)
    W1 = np.asarray(inputs["W1"], np.float32)
    W2 = np.asarray(inputs["W2"], np.float32)
    b1 = np.asarray(inputs["b1"], np.float32)
    gamma = np.asarray(inputs["gamma"], np.float32)
    beta = np.asarray(inputs["beta"], np.float32)
    keys = np.asarray(inputs["keys"], np.float32)
    values = np.ascontiguousarray(np.asarray(inputs["values"], np.float32))

    w1t = np.ascontiguousarray(W1.T)                       # [512, 256]
    w2t = np.ascontiguousarray(W2.T)                       # [256, 1024]
    b1c = np.ascontiguousarray(b1.reshape(F1 // P, P).T)   # [128, 2]
    gamc = np.ascontiguousarray(gamma.reshape(F2 // P, P).T)
    betc = np.ascontiguousarray(beta.reshape(F2 // P, P).T)
    keysT = np.ascontiguousarray(
        keys.reshape(2 * HEADS, SUB, HALF).transpose(0, 2, 1))  # [8, 128, 512]
    iota = np.ascontiguousarray(np.tile(np.arange(KNN, dtype=np.uint16), (P, 1)))
    ident = np.eye(P, dtype=np.float32)

    in_maps = []
    for c in range(n_cores):
        lo, hi = c * b_own, (c + 1) * b_own
        xTc = np.ascontiguousarray(x[lo:hi].T)                    # [512, b_own]
        in_maps.append(dict(
            xT=xTc, w1t=w1t, w2t=w2t, b1c=b1c, gamc=gamc, betc=betc,
            keysT=keysT, vals=values, iota32=iota, ident=ident))
    return in_maps


_CACHE = {}


def kernel(**inputs) -> np.ndarray:
    b_tot = int(inputs["x"].shape[0])
    b_own = b_tot // NCORES
    key = (b_tot, b_own)
    if key not in _CACHE:
        nc = build_program(b_tot, b_own, NCORES)
        _split_matmul_waits(nc)   # HW-only legalization (1 sync wait per PE inst)
        _CACHE[key] = nc
    nc = _CACHE[key]
    in_maps = stage_inputs(inputs, b_tot, b_own, NCORES)
    res = bass_utils.run_bass_kernel_spmd(nc, in_maps, list(range(NCORES)))
    global LAST_RESULTS
    LAST_RESULTS = res
    outs = [r["outT"].T for r in res.results]   # each [b_own, 512]
    return np.ascontiguousarray(np.concatenate(outs, axis=0).astype(np.float32))


LAST_RESULTS = None

